# revision 25
# baseline (speedup 1.0000x reference)
"""Bahdanau additive attention on 8 TRN2 NeuronCores (batch-parallel).

Math: scores[b,i,j] = q[b,i].w + k[b,j].w, masked to -1e9 where mask==0,
softmax over j, then @ value.  The query term q[b,i].w is constant along j,
so it cancels in the softmax:

    out[b,i,:] = (sum_j mask[b,i,j] * e[b,j] * value[b,j,:])
               / (sum_j mask[b,i,j] * e[b,j]),      e[b,j] = exp(k[b,j].w)

(no query needed, no [Lq,Lk] softmax).  Per core: one batch.

Mask encoding: during host-side sharding the 0/1 int32 mask is re-encoded
(losslessly) as fp8_e4m3 bytes (0x38 = 1.0) in a transposed blocked layout

    maskb[it*128 + p, r*128 + i2] = mask[i = it*128 + i2, j = 16*p + r]

so each [128,128] tile arrives with j on partitions and is DIRECTLY a
matmul stationary operand: no PE transposes, no on-device mask conversion,
and 4 MB of mask HBM traffic per core instead of 16 MB.  j-tiles are
mod-16 residue classes (j = 16p + r) matching the key/value chunk layout;
all DMAs are contiguous with >=2KB descriptors.

key/value ship as fp16 (2 MB; |k|,|v| < 6 so range is safe, adds ~3e-4
rel err).  All load DMAs ride the SP (sync) HWDGE ring, interleaved
k/v/mask so the evext chain starts ~3us in; the ACT sequencer only runs
exp + epilogue (a DMA issue costs ~700ns of sequencer time, so v1's 33
ACT-ring DMAs delayed exp_0 to +31us).

Per core pipeline (PE-bound, ~133ns per 257-col matmul at the 2.0 GHz
P0 clock):
  - per residue r: sk_r = k_r.w (DVE stt+accum), e_r = exp(sk) (ACT),
    evext_r = [e*v | e | pad] fp16 (DVE/ACT split).
  - matmul: acc[i] = sum_r maskT(i,r) @ evext_r[:, 0:257], fp8 x fp16,
    i-tiles grouped 7/5/3/1 (one PSUM bank each + 1 warmup bank).
  - epilogue: out_i = acc[:, :256] * (1/acc[:,256]) (DVE recip, mul
    alternating ACT/DVE), stores batched per group on idle DMA paths
    (gpsimd/scalar/gpsimd/sync) so only a 1-tile store trails the last MM.

Dummy matmuls at kernel start trip the PE HAM activity monitor toward
full clock before dense work arrives.
"""

import os
import sys
import types

sys.path.insert(0, "/opt/trn_rl_repo")

import numpy as np

import concourse.bacc as bacc
import concourse.tile as tile
from concourse import mybir
from concourse.bass_utils import run_bass_kernel_spmd


def _ensure_ntff_hook_importable():
    """bass_utils imports antenv.axon_hooks when BASS_TRACE is set; this
    image's antenv lacks that module.  Provide it (and register the real
    ctypes NTFF hook if available) so tracing works instead of crashing."""
    if "antenv.axon_hooks" in sys.modules:
        return
    try:
        import antenv
    except ImportError:
        return
    hooks = types.ModuleType("antenv.axon_hooks")
    hooks._hook = None
    hooks.set_axon_ntff_profile_hook = lambda h: setattr(hooks, "_hook", h)
    hooks.get_axon_ntff_profile_hook = lambda: hooks._hook
    sys.modules["antenv.axon_hooks"] = hooks
    antenv.axon_hooks = hooks
    try:
        from trn_agent_boot.trn_boot import _ntff_profile_via_ctypes

        hook = _ntff_profile_via_ctypes("/opt/axon/libaxon_pjrt.so")
        if hook is not None:
            hooks.set_axon_ntff_profile_hook(hook)
    except Exception:
        pass


_ensure_ntff_hook_importable()

P = 128
B = 8
L = 2048
D = 256
NT = L // P  # 16 chunks/tiles per dim
NE = D + 2  # 258 = value cols + e col + pad (storage); matmuls stream 257
NM = D + 1  # 257 streamed columns
FP8_ONE = 0x38  # fp8_e4m3 1.0

LAST_RESULTS = None


def _build_nc():
    dt = mybir.dt
    nc = bacc.Bacc("TRN2", target_bir_lowering=False, debug=False, num_devices=B)

    key_d = nc.dram_tensor("key", [L, D], dt.float16, kind="ExternalInput").ap()
    value_d = nc.dram_tensor("value", [L, D], dt.float16, kind="ExternalInput").ap()
    maskb_d = nc.dram_tensor("maskb", [L, L], dt.int8, kind="ExternalInput").ap()
    wrep_d = nc.dram_tensor("wrep", [P, D], dt.float32, kind="ExternalInput").ap()
    out_d = nc.dram_tensor("out", [L, D], dt.float32, kind="ExternalOutput").ap()

    with tile.TileContext(nc) as tc:
        with (
            tc.tile_pool(name="const", bufs=1) as const_pool,
            tc.tile_pool(name="big", bufs=1) as big_pool,
            tc.tile_pool(name="small", bufs=1) as small_pool,
            tc.tile_pool(name="junk", bufs=2) as junk_pool,
            tc.tile_pool(name="outp", bufs=2) as out_pool,
            tc.tile_pool(name="rec", bufs=4) as rec_pool,
            tc.tile_pool(name="acc", bufs=7, space="PSUM") as acc_pool,
            tc.tile_pool(name="warm", bufs=1, space="PSUM") as warm_pool,
        ):
            # HAM warmup: dummy matmuls with no real dependencies to bring
            # the PE to full clock before real work arrives (memset on the
            # otherwise-idle GpSimd engine, which clears earliest).
            warm_mv = const_pool.tile([P, 256], dt.float16)
            nc.gpsimd.memset(warm_mv[:], 0.0)
            warm_ps = warm_pool.tile([P, 256], dt.float32)

            def warm(n):
                for _ in range(n):
                    nc.tensor.matmul(
                        warm_ps[:], warm_mv[:, 0:P], warm_mv[:], start=True, stop=True
                    )

            warm(26)

            # --- load DMAs: all on the SP ring, interleaved so k/v quarters
            # land early and mask slabs stream alongside.
            k_big = big_pool.tile([P, NT * D], dt.float16, tag="kbig")
            v_big = big_pool.tile([P, NT * D], dt.float16, tag="vbig")
            mask_all = big_pool.tile([P, NT * L], dt.int8, tag="mask")
            k_view = k_big[:].rearrange("p (t d) -> p t d", d=D)
            v_view = v_big[:].rearrange("p (t d) -> p t d", d=D)
            m_view = mask_all[:].rearrange("p (t c) -> p t c", c=L)
            key_r = key_d.rearrange("(p t) d -> p t d", t=NT)
            value_r = value_d.rearrange("(p t) d -> p t d", t=NT)
            maskb_r = maskb_d.rearrange("(t p) c -> p t c", p=P)
            # wrep rides FIRST on the sync ring: it gates the sk chain, and
            # the ACT HWDGE ring starts its first transfer ~2us later than
            # the sync ring does.
            wrep = const_pool.tile([P, D], dt.float32)
            nc.sync.dma_start(wrep[:], wrep_d[:])

            # k/v quarters interleaved with mask slabs; slabs 0 and 1 ride
            # alone (smaller -> earlier completion sem) so the first
            # matmuls are not mask-gated.
            for q in range(4):
                s = slice(4 * q, 4 * q + 4)
                nc.sync.dma_start(k_view[:, s, :], key_r[:, s, :])
                nc.sync.dma_start(v_view[:, s, :], value_r[:, s, :])
                s2 = slice(2 * q, 2 * q + 2)
                nc.sync.dma_start(m_view[:, s2, :], maskb_r[:, s2, :])
            for q in range(4):
                s2 = slice(8 + 2 * q, 8 + 2 * q + 2)
                nc.sync.dma_start(m_view[:, s2, :], maskb_r[:, s2, :])

            # --- per-chunk prologue: sk_r = k_r.w ; e_r = exp(sk_r) ;
            # evext_r = [e_r * v_r | e_r | pad]  (fp16)
            sk = small_pool.tile([P, NT], dt.float32, tag="sk")
            e_sb = small_pool.tile([P, NT], dt.float32, tag="e")
            evext = big_pool.tile([P, NT * NE], dt.float16, tag="evext")
            ev_v = evext[:].rearrange("p (t n) -> p t n", n=NE)

            def stt(r):
                junk = junk_pool.tile([P, D], dt.float32, tag="junk")
                nc.vector.scalar_tensor_tensor(
                    out=junk[:],
                    in0=k_view[:, r, :],
                    scalar=1.0,
                    in1=wrep[:],
                    op0=mybir.AluOpType.mult,
                    op1=mybir.AluOpType.mult,
                    accum_out=sk[:, r : r + 1],
                )

            for lo, hi in ((0, 4), (4, 8), (8, 12), (12, 16)):
                for r in range(lo, hi):
                    stt(r)
                nc.scalar.activation(
                    e_sb[:, lo:hi], sk[:, lo:hi],
                    mybir.ActivationFunctionType.Exp,
                )
                for r in range(lo, hi):
                    if r % 2 == 0:
                        nc.vector.tensor_scalar_mul(
                            ev_v[:, r, 0:D], v_view[:, r, :], e_sb[:, r : r + 1]
                        )
                    else:
                        nc.scalar.mul(
                            ev_v[:, r, 0:D], v_view[:, r, :], e_sb[:, r : r + 1]
                        )
                nc.vector.tensor_copy(ev_v[:, lo:hi, D], e_sb[:, lo:hi])

            # --- main matmul: acc[i] = sum_r maskT(i, r) @ evext_r
            def stat(i, r):
                off = i * L + r * P
                return mask_all[:, off : off + P].bitcast(dt.float8e4)

            GROUPS = (
                (0, 7, "gpsimd"),
                (7, 12, "scalar"),
                (12, 15, "gpsimd"),
                (15, 16, "sync"),
            )

            def epi_group(g_start, g_end, eng_name, accs):
                n = g_end - g_start
                outt = out_pool.tile(
                    [P, n * D], dt.float32, tag="outt", name=f"outt{g_start}"
                )
                for i in range(g_start, g_end):
                    acc = accs[i]
                    rec = rec_pool.tile([P, 1], dt.float32, tag="rec")
                    nc.vector.reciprocal(rec[:], acc[:, D : D + 1])
                    dst = outt[:, (i - g_start) * D : (i - g_start + 1) * D]
                    if i % 2 == 0:
                        nc.scalar.mul(dst, acc[:, 0:D], rec[:])
                    else:
                        nc.vector.tensor_scalar_mul(dst, acc[:, 0:D], rec[:])
                eng = getattr(nc, eng_name)
                src = outt[:].rearrange("p (t d) -> p t d", d=D)
                dstv = out_d[g_start * P : g_end * P, :].rearrange(
                    "(t p) d -> p t d", p=P
                )
                eng.dma_start(dstv, src)

            for g_start, g_end, eng_name in GROUPS:
                accs = {
                    i: acc_pool.tile([P, NM], dt.float32, tag="acc", name=f"acc{i}")
                    for i in range(g_start, g_end)
                }
                for r in range(NT):
                    if g_start == 0 and r < 6:
                        warm(3)
                    for i in range(g_start, g_end):
                        nc.tensor.matmul(
                            accs[i][:],
                            stat(i, r),
                            ev_v[:, r, 0:NM],
                            start=(r == 0),
                            stop=(r == NT - 1),
                        )
                epi_group(g_start, g_end, eng_name, accs)

    nc.compile()
    return nc


def kernel(query, key, value, mask, w_align):
    global LAST_RESULTS
    key = np.ascontiguousarray(np.asarray(key, dtype=np.float16))
    value = np.ascontiguousarray(np.asarray(value, dtype=np.float16))
    mask = np.asarray(mask)
    w_align = np.asarray(w_align, dtype=np.float32)
    wrep = np.ascontiguousarray(np.tile(w_align[None, :], (P, 1)))

    # Lossless mask re-encode: 0/1 -> fp8_e4m3 {0.0, 1.0} bytes in the
    # transposed blocked layout  maskb[b, it*128+p, r*128+i2] =
    # mask[b, it*128+i2, 16p+r]  (j = 16p + r on partitions).
    m5 = mask.reshape(B, NT, P, P, NT) != 0  # [b, it, i2, p, r]
    maskb = np.where(
        m5.transpose(0, 1, 3, 4, 2), np.int8(FP8_ONE), np.int8(0)
    ).reshape(B, L, L)

    nc = _build_nc()
    in_maps = [
        {"key": key[b], "value": value[b], "maskb": maskb[b], "wrep": wrep}
        for b in range(B)
    ]
    try:
        res = run_bass_kernel_spmd(nc, in_maps, core_ids=list(range(B)))
    except Exception:
        # e.g. trace requested but profiling unavailable -- retry untraced
        os.environ["BASS_NEVER_TRACE"] = "1"
        res = run_bass_kernel_spmd(nc, in_maps, core_ids=list(range(B)))
    LAST_RESULTS = res
    out = np.stack([res.results[b]["out"] for b in range(B)], axis=0)
    return out.astype(np.float32)
